# revision 1
# baseline (speedup 1.0000x reference)
"""Trainium2 Bass kernel for CrossAttention.

Problem shape (hardcoded):
  latent  [8, 4096, 512], context [8, 77, 768]
  wq [512,512], wk/wv [768,512], wo [512,512], biases [512]
  out = softmax((latent@wq+bq)(context@wk+bk)^T / 8) @ (context@wv+bv) @ wo + bo

Sharding: data-parallel over batch — core b handles batch element b.

All matmuls keep the PE contraction dim (partitions) at FULL K=128 coverage:
mixing partial row-group masks between consecutive matmuls hangs/crashes TRN2
(verified empirically), so per-head operands (Dh=64, SKV=77) are stored
head-major and zero-padded up to 128 partitions. Zero rows contribute nothing
to the contraction, and a matmul's cycle cost scales with N only, so the
padding is free in PE time.

Per-core dataflow:
  one-time:
    cT   = context^T (zero-padded)     [128, 6, 77]
    kT_h = 0.125 * (wk_h^T cT + bk_h)  [128(pad 64), 8, 77]   head-major
    v    = cT^T wv + bv                [128(pad 77), 512]
  per 128-row chunk of latent:
    xT    = x^T                        [512, 128]  (PE transpose)
    qT_h  = wq_h^T xT (+bq_h)          [128(pad 64), 8, 128]  head-major
    sT_h  = kT_h^T qT_h                [77, 128]  (scaled scores^T)
    eT_h  = exp(sT_h)                  [128(pad 77), 8, 128]  (no max-sub; |s|<8)
    sums  = ones^T eT_h                [128, 4, 128]  d-major, bcast over parts
    attnT = v_h^T eT_h * 1/sums        [128, 4, 128]  d-major
    out   = attnT^T wo + bo            [128, 512]
"""

import os
import sys
from contextlib import ExitStack

import numpy as np

for _p in ("/opt/trn_rl_repo",):
    if _p not in sys.path and os.path.isdir(_p):
        sys.path.insert(0, _p)

import concourse.bass as bass  # noqa: E402
import concourse.tile as tile  # noqa: E402
from concourse import bacc, mybir  # noqa: E402
from concourse.bass_utils import run_bass_kernel_spmd  # noqa: E402
from concourse.masks import make_identity  # noqa: E402

N_CORES = 8
SQ, D, DC, SKV, H, DH = 4096, 512, 768, 77, 8, 64
F32 = mybir.dt.float32
AF = mybir.ActivationFunctionType

# Matmul operand dtype: 'bf16' (1 cyc/row + FWL) or 'f32' (2x2 half-passes)
MM_DT = os.environ.get("CA_MM_DT", "bf16")
MDT = mybir.dt.bfloat16 if MM_DT == "bf16" else mybir.dt.float32


def _mm(ap):
    return ap


def build_nc(n_chunks=SQ // 128):
    nc = bacc.Bacc("TRN2", target_bir_lowering=False, debug=False)

    lat = nc.dram_tensor("latent", [SQ, D], F32, kind="ExternalInput").ap()
    ctx_d = nc.dram_tensor("context", [SKV, DC], F32, kind="ExternalInput").ap()
    wq = nc.dram_tensor("wq", [D, D], F32, kind="ExternalInput").ap()
    bq = nc.dram_tensor("bq", [D], F32, kind="ExternalInput").ap()
    wk = nc.dram_tensor("wk", [DC, D], F32, kind="ExternalInput").ap()
    bk = nc.dram_tensor("bk", [D], F32, kind="ExternalInput").ap()
    wv = nc.dram_tensor("wv", [DC, D], F32, kind="ExternalInput").ap()
    bv = nc.dram_tensor("bv", [D], F32, kind="ExternalInput").ap()
    wo = nc.dram_tensor("wo", [D, D], F32, kind="ExternalInput").ap()
    bo = nc.dram_tensor("bo", [D], F32, kind="ExternalInput").ap()
    out_d = nc.dram_tensor("out", [SQ, D], F32, kind="ExternalOutput").ap()

    with tile.TileContext(nc) as tc:
        with ExitStack() as stk:
            consts = stk.enter_context(tc.tile_pool(name="consts", bufs=1))
            prep = stk.enter_context(tc.tile_pool(name="prep", bufs=1))
            xpool = stk.enter_context(tc.tile_pool(name="x", bufs=4))
            spool = stk.enter_context(tc.tile_pool(name="work", bufs=4))
            opool = stk.enter_context(tc.tile_pool(name="outp", bufs=4))
            pspool = stk.enter_context(
                tc.tile_pool(name="psA", bufs=3, space="PSUM")
            )
            psB = stk.enter_context(
                tc.tile_pool(name="psB", bufs=5, space="PSUM")
            )

            def ps_tile(name):
                return psB.tile([128, 4, 128], F32, tag="psB", name=name)

            def load_w(ap_in, shape, name):
                t = consts.tile(shape, MDT, name=name)
                if MDT == F32:
                    nc.sync.dma_start(t, ap_in)
                else:
                    nc.gpsimd.dma_start(t, ap_in)  # SWDGE casts f32 -> bf16
                return t

            # ---------------- constants ----------------
            wq_sb = load_w(wq.rearrange("(t p) d -> p t d", p=128), [128, 4, D], "wq_sb")
            wk_sb = load_w(wk.rearrange("(t p) d -> p t d", p=128), [128, 6, D], "wk_sb")
            wv_sb = load_w(wv.rearrange("(t p) d -> p t d", p=128), [128, 6, D], "wv_sb")
            wo_sb = load_w(wo.rearrange("(t p) d -> p t d", p=128), [128, 4, D], "wo_sb")

            # bq in row-0-padded layout for the PE rank-1 bias matmul
            bq_pad = consts.tile([128, D], MDT, name="bq_pad")
            nc.vector.memset(bq_pad, 0.0)
            (nc.sync if MDT == F32 else nc.gpsimd).dma_start(bq_pad[0:1, :], bq.rearrange("(o d) -> o d", o=1))
            bk_hm = consts.tile([64, H], F32, name="bk_hm")
            nc.sync.dma_start(bk_hm, bk.rearrange("(h p) -> p h", p=64))
            bk_hms = consts.tile([64, H], F32, name="bk_hms")
            nc.vector.tensor_scalar_mul(bk_hms, bk_hm, 0.125)

            # row-0 padded biases for K=128 rank-1 bias matmuls
            bv_pad = consts.tile([128, D], MDT, name="bv_pad")
            nc.vector.memset(bv_pad, 0.0)
            (nc.sync if MDT == F32 else nc.gpsimd).dma_start(bv_pad[0:1, :], bv.rearrange("(o d) -> o d", o=1))
            bo_pad = consts.tile([128, D], MDT, name="bo_pad")
            nc.vector.memset(bo_pad, 0.0)
            (nc.sync if MDT == F32 else nc.gpsimd).dma_start(bo_pad[0:1, :], bo.rearrange("(o d) -> o d", o=1))
            # e0 [128,128]: row 0 all-ones, rest zero (lhsT of bias matmuls)
            e0 = consts.tile([128, 128], MDT, name="e0")
            nc.vector.memset(e0, 0.0)
            nc.vector.memset(e0[0:1, :], 1.0)
            # ones on rows < SKV, zeros below (lhsT of softmax-sum matmuls)
            ones_kv = consts.tile([128, DH], MDT, name="ones_kv")
            nc.vector.memset(ones_kv, 0.0)
            nc.vector.memset(ones_kv[:64, :], 1.0)
            nc.vector.memset(ones_kv[64:SKV, :], 1.0)
            ident = consts.tile([128, 128], MDT, name="ident")
            make_identity(nc, ident)
            zeros_sb = consts.tile([64, H, 128], MDT, name="zeros_sb")
            nc.vector.memset(zeros_sb, 0.0)

            # ---------------- K/V prep (once) ----------------
            ctx_sb = prep.tile([128, DC], MDT, name="ctx_sb")
            nc.vector.memset(ctx_sb, 0.0)
            (nc.sync if MDT == F32 else nc.gpsimd).dma_start(ctx_sb[:SKV, :], ctx_d)
            # cT zero-padded beyond kv=77 comes out of transposing zero rows
            cT_sb = prep.tile([128, 6, SKV], MDT, name="cT_sb")
            for g in range(2):
                cT_ps = psB.tile([128, 3, 128], MDT, tag="psB", name="cT_ps")
                for t3 in range(3):
                    t = g * 3 + t3
                    nc.tensor.transpose(
                        cT_ps[:, t3, :], ctx_sb[:, t * 128 : (t + 1) * 128], ident
                    )
                nc.vector.tensor_copy(cT_sb[:, 3 * g : 3 * g + 3, :], cT_ps[:, :, :SKV])

            # kT head-major, rows >= 64 zero
            kT_sb = prep.tile([128, H, SKV], MDT, name="kT_sb")
            nc.vector.memset(kT_sb[64:128, :, :], 0.0)
            for g in range(2):
                kT_ps = psB.tile([64, 4, SKV], F32, tag="psB", name="kT_ps")
                for hi in range(4):
                    h = g * 4 + hi
                    for ct in range(6):
                        nc.tensor.matmul(
                            kT_ps[:, hi, :],
                            lhsT=_mm(wk_sb[:, ct, h * 64 : (h + 1) * 64]),
                            rhs=_mm(cT_sb[:, ct, :]),
                            start=(ct == 0),
                            stop=(ct == 5),
                        )
                for hi in range(4):
                    h = g * 4 + hi
                    nc.scalar.activation(
                        kT_sb[:64, h, :],
                        kT_ps[:, hi, :],
                        AF.Identity,
                        bias=bk_hms[:, h : h + 1],
                        scale=0.125,
                    )

            # v zero-padded beyond kv=77
            v_ps = psB.tile([128, 4, 128], F32, tag="psB", name="v_ps")
            for ct in range(6):
                nc.tensor.matmul(
                    v_ps[:SKV, :, :].rearrange("p a b -> p (a b)"),
                    lhsT=_mm(cT_sb[:, ct, :]),
                    rhs=_mm(wv_sb[:, ct, :]),
                    start=(ct == 0),
                    stop=False,
                )
            nc.tensor.matmul(
                v_ps[:SKV, :, :].rearrange("p a b -> p (a b)"),
                lhsT=_mm(e0[:, :SKV]),
                rhs=_mm(bv_pad),
                start=False,
                stop=True,
            )
            v_sb = prep.tile([128, D], MDT, name="v_sb")
            nc.vector.memset(v_sb[64:128, :], 0.0)
            nc.vector.tensor_copy(
                v_sb[:SKV, :], v_ps[:SKV, :, :].rearrange("p a b -> p (a b)")
            )

            # ---------------- main loop: 2-stage software pipeline ----------
            # stageA(ci): load+transpose+project -> qT_sb; stageB(ci): softmax
            # +PV+out_proj. Emitting A(i+1) before B(i) lets the PE chew the
            # next chunk's projection while ACT/DVE run this chunk's softmax.
            stage_state = {}

            def stageA(ci):
                r0 = ci * 128
                x_sb = xpool.tile([128, D], MDT, tag="x", name="x_sb")
                (nc.sync if MDT == F32 else nc.gpsimd).dma_start(x_sb, lat[r0 : r0 + 128, :])

                xT_ps = pspool.tile([128, 4, 128], MDT, tag="psA", name="xT_ps")
                for et in range(4):
                    nc.tensor.transpose(
                        xT_ps[:, et, :], x_sb[:, et * 128 : (et + 1) * 128], ident
                    )
                xT_sb = spool.tile([128, 4, 128], MDT, tag="xT", name="xT_sb")
                nc.scalar.copy(xT_sb, xT_ps)

                # qT head-major [64, 8, 128] in psum (two 1-bank tiles)
                qT_sb = spool.tile([128, H, 128], MDT, tag="qT", name="qT_sb")
                nc.vector.tensor_copy(qT_sb[64:128, :, :], zeros_sb)
                for g in range(2):
                    qT_ps = pspool.tile([64, 4, 128], F32, tag="psA", name="qT_ps")
                    for hi in range(4):
                        h = g * 4 + hi
                        for et in range(4):
                            nc.tensor.matmul(
                                qT_ps[:, hi, :],
                                lhsT=_mm(wq_sb[:, et, h * 64 : (h + 1) * 64]),
                                rhs=_mm(xT_sb[:, et, :]),
                                start=(et == 0),
                                stop=False,
                            )
                        nc.tensor.matmul(
                            qT_ps[:, hi, :],
                            lhsT=_mm(bq_pad[:, h * 64 : (h + 1) * 64]),
                            rhs=_mm(e0),
                            start=False,
                            stop=True,
                        )
                    nc.vector.tensor_copy(
                        qT_sb[:64, g * 4 : g * 4 + 4, :], qT_ps
                    )

                stage_state[ci] = qT_sb

            def stageB(ci):
                r0 = ci * 128
                qT_sb = stage_state.pop(ci)
                expT_sb = spool.tile([128, H, 128], MDT, tag="expT", name="expT_sb")
                nc.vector.tensor_copy(expT_sb[64:128, :, :], zeros_sb)
                for g in range(2):
                    sT_ps = ps_tile("sT_ps")
                    for hi in range(4):
                        h = g * 4 + hi
                        nc.tensor.matmul(
                            sT_ps[:SKV, hi, :],
                            lhsT=_mm(kT_sb[:, h, :]),
                            rhs=_mm(qT_sb[:, h, :]),
                            start=True,
                            stop=True,
                        )
                    nc.scalar.activation(
                        expT_sb[:SKV, g * 4 : g * 4 + 4, :],
                        sT_ps[:SKV, :, :],
                        AF.Exp,
                    )

                sums_ps = ps_tile("sums_ps")
                for h in range(H):
                    dt, off = h // 2, (h % 2) * 64
                    nc.tensor.matmul(
                        sums_ps[off : off + 64, dt, :],
                        lhsT=_mm(ones_kv),
                        rhs=_mm(expT_sb[:, h, :]),
                        start=True,
                        stop=True,
                    )
                rsum_sb = spool.tile([128, 4, 128], F32, tag="rsum", name="rsum_sb")
                nc.vector.reciprocal_approx_fast(rsum_sb, sums_ps)

                attnT_ps = ps_tile("attnT_ps")
                for h in range(H):
                    dt, off = h // 2, (h % 2) * 64
                    nc.tensor.matmul(
                        attnT_ps[off : off + 64, dt, :],
                        lhsT=_mm(v_sb[:, h * 64 : (h + 1) * 64]),
                        rhs=_mm(expT_sb[:, h, :]),
                        start=True,
                        stop=True,
                    )
                attnT_sb = spool.tile([128, 4, 128], MDT, tag="attnT", name="attnT_sb")
                nc.vector.tensor_mul(attnT_sb, attnT_ps, rsum_sb)

                out_ps = ps_tile("out_ps")
                out_flat = out_ps.rearrange("p a b -> p (a b)")
                for dt in range(4):
                    nc.tensor.matmul(
                        out_flat,
                        lhsT=_mm(attnT_sb[:, dt, :]),
                        rhs=_mm(wo_sb[:, dt, :]),
                        start=(dt == 0),
                        stop=False,
                    )
                nc.tensor.matmul(
                    out_flat, lhsT=_mm(e0), rhs=_mm(bo_pad), start=False, stop=True
                )
                out_sb = opool.tile([128, D], F32, tag="out", name="out_sb")
                nc.scalar.copy(out_sb, out_flat)
                nc.sync.dma_start(out_d[r0 : r0 + 128, :], out_sb)

            stageA(0)
            for ci in range(1, n_chunks):
                stageA(ci)
                stageB(ci - 1)
            stageB(n_chunks - 1)

    nc.compile()
    return nc


_BUILD_CACHE = {}


def _get_nc():
    key = (MM_DT,)
    if key not in _BUILD_CACHE:
        _BUILD_CACHE[key] = build_nc()
    return _BUILD_CACHE[key]


def _in_maps(latent, context, wq, bq, wk, bk, wv, bv, wo, bo):
    f = lambda a: np.ascontiguousarray(np.asarray(a), dtype=np.float32)
    shared = {
        "wq": f(wq), "bq": f(bq), "wk": f(wk), "bk": f(bk),
        "wv": f(wv), "bv": f(bv), "wo": f(wo), "bo": f(bo),
    }
    maps = []
    for b in range(N_CORES):
        m = dict(shared)
        m["latent"] = f(latent[b])
        m["context"] = f(context[b])
        maps.append(m)
    return maps


def run_on_hw(inputs, trace=False, **kw):
    nc = _get_nc()
    maps = _in_maps(**inputs)
    res = run_bass_kernel_spmd(nc, maps, list(range(N_CORES)), trace=trace, **kw)
    out = np.stack([res.results[b]["out"] for b in range(N_CORES)], axis=0)
    return out, res


def kernel(latent, context, wq, bq, wk, bk, wv, bv, wo, bo):
    out, _ = run_on_hw(dict(
        latent=latent, context=context, wq=wq, bq=bq, wk=wk, bk=bk,
        wv=wv, bv=bv, wo=wo, bo=bo,
    ))
    return out

